# revision 1
# baseline (speedup 1.0000x reference)
"""CoLightAgent forward kernel for 8 Trainium2 NeuronCores.

Math note: in the reference, ne = broadcast(adj @ emb) over the agent axis i,
so nh.sum(axis=3) / hid.sum(axis=3) are independent of i and collapse to
per-batch vectors S_n, S_h of shape [T].  The final gather keeps only row
tgt[b] of the agent branch.  The whole [B,N,N,T] intermediate disappears:

    E    = relu(relu(obs @ We1 + be1) @ We2 + be2)        # [N, T] per batch
    AE   = adj @ E                                        # [N, T]
    S_n  = sum_j relu(AE @ Wn + bn)[j, :]                 # [T]
    S_h  = sum_j relu(AE @ Wh + bh)[j, :]                 # [T]
    a    = relu(E[tgt] @ Wl + bl)                         # [T]
    attn = softmax_d((a * S_n).reshape(D, H).T)           # [H, D]
    g    = mean_h(attn * S_h.reshape(D, H).T)             # [D]
    act  = g @ Wa + ba                                    # [ACT]

Sharding: data-parallel over the batch; core c computes batch c % 4 in full
(cores 4..7 duplicate 0..3 and their outputs are ignored).  All matmuls keep
the contraction dim on partitions; activations flow as
    E1T [t, n] -> E [n, t'] -> AET [t, m]
so every stage feeds the next as lhsT/rhs without transposes.

The softmax runs entirely in the [T, 1] column layout: logits l = a*S_n are
>= 0 (product of relu outputs), so exp(min(l, 85)) never over/underflows and
matches the reference's max-subtracted softmax to fp32 accuracy.  Per-head
sums / broadcasts use tiny 0/1 selector matmuls (Gh, Gh^T, Gd) instead of a
DRAM round-trip for the (d h) -> h d regrouping.

Inputs are packed host-side into 8 DMAs issued from two HWDGE queues
(SP + ACT), ordered so each tensor lands just before the stage that needs
it.  The N=256 matmuls run in fp32r mode (4x the fp32 rate, verified on HW
at ~7e-5 relative error); the N=1 matmuls stay plain fp32.
"""

import numpy as np

import concourse.bacc as bacc
import concourse.mybir as mybir
import concourse.tile as tile
from concourse import bass_utils
from concourse.bass import ts

B, N, OBS, ACT = 4, 256, 40, 8
HEAD, DIM = 8, 32
T = HEAD * DIM
P = 128
F32 = mybir.dt.float32
F32R = mybir.dt.float32r
AF = mybir.ActivationFunctionType
AX = mybir.AxisListType
MULT = mybir.AluOpType.mult
CLAMP = 85.0

_CACHE = {}


def _build_nc():
    nc = bacc.Bacc("TRN2", target_bir_lowering=False, debug=False, num_devices=8)

    d_small = nc.dram_tensor("pk_small", [P, 2 * N + 48], F32R, kind="ExternalInput")
    d_tiny = nc.dram_tensor("pk_tiny", [1, 384], F32R, kind="ExternalInput")
    d_we2 = nc.dram_tensor("pk_we2", [2 * P, N], F32R, kind="ExternalInput")
    d_adjt = nc.dram_tensor("pk_adjt", [2 * P, N], F32R, kind="ExternalInput")
    d_wn = nc.dram_tensor("pk_wn", [2 * P, N], F32R, kind="ExternalInput")
    d_wl = nc.dram_tensor("pk_wl", [2 * P, N], F32, kind="ExternalInput")
    d_wh = nc.dram_tensor("pk_wh", [2 * P, N], F32R, kind="ExternalInput")
    d_sel = nc.dram_tensor("pk_sel", [2 * P, N], F32, kind="ExternalInput")
    d_out = nc.dram_tensor("act_out", [ACT], F32, kind="ExternalOutput")

    with tile.TileContext(nc) as tc:
        with (
            tc.tile_pool(name="w", bufs=1) as wp,
            tc.tile_pool(name="work", bufs=2) as work,
            tc.tile_pool(name="mmps", bufs=4, space="PSUM") as ps,
            tc.tile_pool(name="vecps", bufs=2, space="PSUM") as psv,
            tc.tile_pool(name="smps", bufs=1, space="PSUM") as pss,
        ):
            # ---- staged inputs --------------------------------------------
            small_t = wp.tile([P, 2 * N + 48], F32R)
            tiny_t = wp.tile([1, 384], F32R)
            we2_t = wp.tile([P, 2, N], F32R)
            adjt_t = wp.tile([P, 2, N], F32R)
            wn_t = wp.tile([P, 2, N], F32R)
            wl_t = wp.tile([P, 2, N], F32)
            wh_t = wp.tile([P, 2, N], F32R)
            sel_t = wp.tile([P, 2, N], F32)
            rq = lambda d: d.ap().rearrange("(q p) m -> p q m", p=P)
            nc.sync.dma_start(small_t[:], d_small.ap())
            nc.scalar.dma_start(we2_t[:], rq(d_we2))
            nc.sync.dma_start(tiny_t[:], d_tiny.ap())
            nc.scalar.dma_start(adjt_t[:], rq(d_adjt))
            nc.sync.dma_start(wn_t[:], rq(d_wn))
            nc.sync.dma_start(wl_t[:], rq(d_wl))
            nc.sync.dma_start(wh_t[:], rq(d_wh))
            nc.sync.dma_start(sel_t[:], rq(d_sel))
            bias_t = small_t[:, 2 * N:2 * N + 48].rearrange(
                "p (s c) -> p s c", c=24).bitcast(F32)

            be1 = lambda s: bias_t[:, s, 0:1]
            bl = lambda s: bias_t[:, s, 1:2]
            bn = lambda s: bias_t[:, s, 2:3]
            bh = lambda s: bias_t[:, s, 3:4]
            oh = lambda s: bias_t[:, s, 4:5]
            ba = bias_t[0:ACT, 0, 5:6]
            Gh = lambda s: bias_t[:, s, 8:16]
            Wa = bias_t[:, 0, 16:24]
            Wl = lambda s: wl_t[:, s, :]
            GhT = sel_t[:, 0, :]
            Wbig = lambda s: sel_t[:, 1, ts(s, ACT)]

            obsT_r = small_t[:, 0:N]
            We1_r = small_t[:, N:2 * N]
            We2r = lambda s: we2_t[:, s, :]
            adjTr = lambda s: adjt_t[:, s, :]
            Wnr = lambda s: wn_t[:, s, :]
            Whr = lambda s: wh_t[:, s, :]
            ones1 = tiny_t[0:1, 0:P]
            be2r = tiny_t[0:1, P:P + N]

            zeros_t = wp.tile([P, N], F32)
            nc.vector.memset(zeros_t[:], 0.0)

            # ---- stage 1: E1T[t, n] = relu(We1.T @ obsT + be1) ------------
            E1T_t = wp.tile([P, 2, N], F32R)
            for s in range(2):
                pm = ps.tile([P, N], F32, tag="mm")
                nc.tensor.matmul(pm[:], We1_r[:, ts(s, P)], obsT_r,
                                 start=True, stop=True)
                nc.vector.scalar_tensor_tensor(
                    E1T_t[:, s, :], pm[:], be1(s), zeros_t[:],
                    mybir.AluOpType.add, mybir.AluOpType.max)

            # ---- stage 2: E[n, t'] = relu(E1 @ We2 + be2) -----------------
            E_t = wp.tile([P, 2, T], F32R)
            for s in range(2):
                pm = ps.tile([P, T], F32, tag="mm")
                nc.tensor.matmul(pm[:], E1T_t[:, 0, ts(s, P)], We2r(0),
                                 start=True, stop=False)
                nc.tensor.matmul(pm[:], E1T_t[:, 1, ts(s, P)], We2r(1),
                                 start=False, stop=False)
                nc.tensor.matmul(pm[:], ones1, be2r, start=False, stop=True)
                if s == 0:
                    nc.vector.tensor_scalar_max(E_t[:, s, :], pm[:], 0.0)
                else:
                    nc.scalar.activation(E_t[:, s, :], pm[:], AF.Relu)

            # ---- stage 3: AET[t, m] = (adj @ E).T = E-as-lhsT @ adjT ------
            AET_t = wp.tile([P, 2, N], F32R)
            for s in range(2):
                pm = ps.tile([P, N], F32, tag="mm")
                nc.tensor.matmul(pm[:], E_t[:, 0, ts(s, P)], adjTr(0),
                                 start=True, stop=False)
                nc.tensor.matmul(pm[:], E_t[:, 1, ts(s, P)], adjTr(1),
                                 start=False, stop=True)
                if s == 0:
                    nc.vector.tensor_copy(AET_t[:, s, :], pm[:])
                else:
                    nc.scalar.copy(AET_t[:, s, :], pm[:])

            # ---- stages 4/5: S_n / S_h (relu + row-sum fused) -------------
            # S_n gates the softmax chain, so its two slices run in parallel
            # on ACT and DVE; S_h (only needed later by v) follows.
            Sn_t = wp.tile([P, 2, 1], F32)
            Sh_t = wp.tile([P, 2, 1], F32)

            def relu_rowsum(W, bv, S_t, act_after=None, dve_after=None):
                for s in range(2):
                    pm = ps.tile([P, N], F32, tag="mm")
                    nc.tensor.matmul(pm[:], W(0)[:, ts(s, P)], AET_t[:, 0, :],
                                     start=True, stop=False)
                    nc.tensor.matmul(pm[:], W(1)[:, ts(s, P)], AET_t[:, 1, :],
                                     start=False, stop=True)
                    zt = work.tile([P, N], F32, tag="zsc")
                    if s == 0:
                        bi = nc.scalar.activation(zt[:], pm[:], AF.Relu,
                                                  bias=bv(s),
                                                  accum_out=S_t[:, s, :])
                        if act_after is not None:
                            tile.add_dep_helper(act_after.ins, bi.ins,
                                                sync=False,
                                                reason="yield ACT to softmax")
                    else:
                        bi = nc.vector.scalar_tensor_tensor(
                            zt[:], pm[:], bv(s), zeros_t[:],
                            mybir.AluOpType.add, mybir.AluOpType.max,
                            accum_out=S_t[:, s, :])
                        if dve_after is not None:
                            tile.add_dep_helper(dve_after.ins, bi.ins,
                                                sync=False,
                                                reason="yield DVE to softmax")

            relu_rowsum(Wnr, bn, Sn_t)

            # ---- stage 6: e_t = E[tgt, :] via one-hot ---------------------
            et_t = wp.tile([P, 2, 1], F32)
            Ef = E_t[:].bitcast(F32)
            for s in range(2):
                pv = psv.tile([P, 1], F32, tag="vec")
                nc.tensor.matmul(pv[:], Ef[:, 0, ts(s, P)], oh(0),
                                 start=True, stop=False)
                nc.tensor.matmul(pv[:], Ef[:, 1, ts(s, P)], oh(1),
                                 start=False, stop=True)
                nc.vector.tensor_copy(et_t[:, s, :], pv[:])

            # ---- stage 7: a = relu(Wl.T @ e_t + bl) -----------------------
            a_t = wp.tile([P, 2, 1], F32)
            for s in range(2):
                pv = psv.tile([P, 1], F32, tag="vec")
                nc.tensor.matmul(pv[:], Wl(0)[:, ts(s, P)], et_t[:, 0, :],
                                 start=True, stop=False)
                nc.tensor.matmul(pv[:], Wl(1)[:, ts(s, P)], et_t[:, 1, :],
                                 start=False, stop=True)
                nc.scalar.activation(a_t[:, s, :], pv[:], AF.Relu, bias=bl(s))

            # ---- softmax epilogue, all in [T, 1] column layout ------------
            # l = min(a * S_n, CLAMP); expl = exp(l)  (l >= 0 always)
            l_t = wp.tile([P, 2], F32)
            expl_t = wp.tile([P, 2], F32)
            nc.vector.tensor_mul(l_t[:], a_t[:, :, 0], Sn_t[:, :, 0])
            min_bi = nc.vector.tensor_scalar_min(l_t[:], l_t[:], CLAMP)
            exp_bi = nc.scalar.activation(expl_t[:], l_t[:], AF.Exp)

            relu_rowsum(Whr, bh, Sh_t, act_after=exp_bi, dve_after=min_bi)

            # denom[h] = sum_d expl[d*8+h];  recip = 1/denom
            recip_t = wp.tile([P, 1], F32)
            nc.vector.memset(recip_t[:], 0.0)
            pd = pss.tile([HEAD, 1], F32, tag="tiny")
            nc.tensor.matmul(pd[:], Gh(0), expl_t[:, 0:1], start=True, stop=False)
            nc.tensor.matmul(pd[:], Gh(1), expl_t[:, 1:2], start=False, stop=True)
            nc.vector.reciprocal(recip_t[0:HEAD, :], pd[:])

            # v[t] = expl[t] * recip[t%8] * S_h[t]
            # act[a] = sum_t Wbig[t, a] * v[t]   (Wbig[t,a] = Wa[t//8,a]/8)
            v_t = wp.tile([P, 2], F32)
            pa = pss.tile([ACT, 1], F32, tag="tiny")
            for s in range(2):
                pv = psv.tile([P, 1], F32, tag="vec")
                nc.tensor.matmul(pv[:], GhT[:, ts(s, P)], recip_t[:],
                                 start=True, stop=True)
                nc.vector.scalar_tensor_tensor(v_t[:, s:s + 1], expl_t[:, s:s + 1],
                                               pv[:], Sh_t[:, s, :], MULT, MULT)
            nc.tensor.matmul(pa[:], Wbig(0), v_t[:, 0:1], start=True, stop=False)
            nc.tensor.matmul(pa[:], Wbig(1), v_t[:, 1:2], start=False, stop=True)
            res_t = wp.tile([ACT, 1], F32)
            nc.scalar.activation(res_t[:], pa[:], AF.Identity, bias=ba, scale=1.0)
            nc.sync.dma_start(d_out.ap(), res_t[:, 0])

    nc.compile()
    return nc


def get_nc():
    if "nc" not in _CACHE:
        _CACHE["nc"] = _build_nc()
    return _CACHE["nc"]


def _selectors():
    t = np.arange(T)
    Gh = (t[:, None] % HEAD == np.arange(HEAD)[None, :]).astype(np.float32)
    GhT_pad = np.zeros((P, T), np.float32)
    GhT_pad[:HEAD, :] = Gh.T
    return Gh, GhT_pad


def make_in_maps(x, adj, We1, be1, We2, be2, Wl, bl, Wn, bn, Wh, bh, Wa, ba):
    f = lambda v: np.ascontiguousarray(np.asarray(v, np.float32))
    x = f(x)
    tgt = x[:, -1, 0].astype(np.int32)
    obs = x[:, :-1, :]
    Gh, GhT_pad = _selectors()

    pk_bias = np.zeros((N, 24), np.float32)
    pk_bias[:, 0] = np.asarray(be1, np.float32)
    pk_bias[:, 1] = np.asarray(bl, np.float32)
    pk_bias[:, 2] = np.asarray(bn, np.float32)
    pk_bias[:, 3] = np.asarray(bh, np.float32)
    pk_bias[:ACT, 5] = np.asarray(ba, np.float32)
    pk_bias[:, 8:16] = Gh
    pk_bias[:DIM, 16:24] = np.asarray(Wa, np.float32)


    pk_tiny = np.zeros((1, 384), np.float32)
    pk_tiny[0, 0:P] = 1.0
    pk_tiny[0, P:P + N] = np.asarray(be2, np.float32)
    pk_we2 = f(We2)
    pk_adjt = np.ascontiguousarray(f(np.asarray(adj).T))
    pk_wn = f(Wn)
    pk_wl = f(Wl)
    pk_wh = f(Wh)
    WbigSlab = np.zeros((P, N), np.float32)
    Wa8 = np.asarray(Wa, np.float32) / HEAD
    for si in range(2):
        tt = si * P + np.arange(P)
        WbigSlab[:, si * ACT:(si + 1) * ACT] = Wa8[tt // HEAD, :]
    pk_sel = np.ascontiguousarray(np.concatenate([GhT_pad, WbigSlab], axis=0))

    in_maps = []
    for c in range(8):
        b = c % B
        pb = pk_bias.copy()
        pb[tgt[b], 4] = 1.0
        in_maps.append({
            "pk_small": np.ascontiguousarray(np.concatenate(
                [np.pad(obs[b].T, ((0, P - OBS), (0, 0))),
                 np.pad(f(We1), ((0, P - OBS), (0, 0))),
                 pb.reshape(2, P, 24).transpose(1, 0, 2).reshape(P, 48),
                 ], axis=1)),
            "pk_tiny": pk_tiny,
            "pk_we2": pk_we2,
            "pk_adjt": pk_adjt,
            "pk_wn": pk_wn,
            "pk_wl": pk_wl,
            "pk_wh": pk_wh,
            "pk_sel": pk_sel,
        })
    return in_maps


def run(in_maps, **kwargs):
    nc = get_nc()
    return bass_utils.run_bass_kernel_spmd(
        nc, in_maps, core_ids=list(range(8)), **kwargs)


def kernel(**inputs) -> np.ndarray:
    in_maps = make_in_maps(**inputs)
    res = run(in_maps)
    return np.stack(
        [res.results[b]["act_out"] for b in range(B)], axis=0).astype(np.float32)



# revision 19
# speedup vs baseline: 1.0768x; 1.0768x over previous
"""CoLightAgent forward kernel for 8 Trainium2 NeuronCores.

Math note: in the reference, ne = broadcast(adj @ emb) over the agent axis i,
so nh.sum(axis=3) / hid.sum(axis=3) are independent of i and collapse to
per-batch vectors S_n, S_h of shape [T].  The final gather keeps only row
tgt[b] of the agent branch.  The whole [B,N,N,T] intermediate disappears:

    E    = relu(relu(obs @ We1 + be1) @ We2 + be2)        # [N, T] per batch
    AE   = adj @ E                                        # [N, T]
    S_n  = sum_j relu(AE @ Wn + bn)[j, :]                 # [T]
    S_h  = sum_j relu(AE @ Wh + bh)[j, :]                 # [T]
    a    = relu(E[tgt] @ Wl + bl)                         # [T]
    attn = softmax_d((a * S_n).reshape(D, H).T)           # [H, D]
    g    = mean_h(attn * S_h.reshape(D, H).T)             # [D]
    act  = g @ Wa + ba                                    # [ACT]

Sharding: data-parallel over the batch; core c computes batch c % 4 in full
(cores 4..7 duplicate 0..3 and their outputs are ignored).

All heavy tensors travel and multiply in bf16 (validated ~2e-3 rel err vs
the fp32 reference, against a 2e-2 budget), which halves DMA traffic.  The
inputs arrive in 5 HWDGE DMAs (SP queue) + 2 SWDGE DMAs (Pool queue) so the
two descriptor-generation paths run in parallel; payloads are ordered so
each tensor lands just before the stage that needs it.

Biases are folded into the systolic array: be1 rides as a 41st contraction
row of the stage-1 matmul, and be2/bl/bn/bh/ba are applied with rank-1
[1,128]x[1,N] matmuls against an on-chip memset ones-row, so no fp32 bias
block or biased activations are needed.  The softmax denominator broadcast
uses a [128,128] 0/1 matrix M (M[i,j] = i%8==j%8), giving per-partition
reciprocals directly.
"""

import numpy as np
import ml_dtypes

import concourse.bacc as bacc
import concourse.mybir as mybir
import concourse.tile as tile
from concourse import bass_utils
from concourse.bass import ts

B, N, OBS, ACT = 4, 256, 40, 8
HEAD, DIM = 8, 32
T = HEAD * DIM
P = 128
F32 = mybir.dt.float32
BF16 = mybir.dt.bfloat16
AF = mybir.ActivationFunctionType
ALU = mybir.AluOpType
CLAMP = 85.0
BF = ml_dtypes.bfloat16

_CACHE = {}

A1_COLS = 528   # 256 obsT | 256 We1 | 8 ba | 8 pad   (rows 0:40 data, row 40 bias)
A2_COLS = 1024  # one row: 256 be2 | 256 bl | 256 bn | 256 bh
ADJ_COLS = 516  # 512 adjT | 2 oh | 2 pad
E_COLS = 664    # 512 Wl | 128 M | 16 Wbig | 8 pad


def _build_nc():
    nc = bacc.Bacc("TRN2", target_bir_lowering=False, debug=False, num_devices=8)

    d_a1 = nc.dram_tensor("pk_a1", [48, A1_COLS], BF16, kind="ExternalInput")
    d_a2 = nc.dram_tensor("pk_a2", [1, A2_COLS], BF16, kind="ExternalInput")
    d_we2 = nc.dram_tensor("pk_we2", [P, 512], BF16, kind="ExternalInput")
    d_adjt = nc.dram_tensor("pk_adjt", [P, ADJ_COLS], BF16, kind="ExternalInput")
    d_wn = nc.dram_tensor("pk_wn", [P, 512], BF16, kind="ExternalInput")
    d_wh = nc.dram_tensor("pk_wh", [P, 512], BF16, kind="ExternalInput")
    d_e = nc.dram_tensor("pk_e", [P, E_COLS], BF16, kind="ExternalInput")
    d_out = nc.dram_tensor("act_out", [P], F32, kind="ExternalOutput")

    with tile.TileContext(nc) as tc:
        with (
            tc.tile_pool(name="w", bufs=1) as wp,
            tc.tile_pool(name="work", bufs=2) as work,
            tc.tile_pool(name="mmps", bufs=4, space="PSUM") as ps,
            tc.tile_pool(name="vecps", bufs=2, space="PSUM") as psv,
            tc.tile_pool(name="smps", bufs=1, space="PSUM") as pss,
        ):
            a1_t = wp.tile([48, A1_COLS], BF16)
            a2_t = wp.tile([1, A2_COLS], BF16)
            we2_t = wp.tile([P, 512], BF16)
            adjt_t = wp.tile([P, ADJ_COLS], BF16)
            wn_t = wp.tile([P, 512], BF16)
            wh_t = wp.tile([P, 512], BF16)
            e_t = wp.tile([P, E_COLS], BF16)

            # --- input DMAs: SP queue -> HWDGE; Pool queue -> SWDGE --------
            nc.sync.dma_start(a1_t[:], d_a1.ap())        # h0: obsT/We1/be1/ba
            nc.sync.dma_start(a2_t[:], d_a2.ap())        # h1: bias rows (tiny)
            nc.gpsimd.dma_start(we2_t[:], d_we2.ap())    # p0: We2
            nc.sync.dma_start(adjt_t[:], d_adjt.ap())    # h2: adjT + oh
            nc.gpsimd.dma_start(wn_t[:], d_wn.ap())      # p1: Wn
            nc.sync.dma_start(e_t[:], d_e.ap())          # h3: Wl/M/Wbig
            nc.sync.dma_start(wh_t[:], d_wh.ap())        # h4: Wh

            # views
            obsT = a1_t[0:41, 0:256]                      # rows 40 = ones (be1)
            We1a = lambda s: a1_t[0:41, 256 + s * P:256 + (s + 1) * P]
            ba_row = a1_t[0:1, 512:520]
            be2_row = a2_t[0:1, 0:256]
            bl_row = lambda s: a2_t[0:1, 256 + s * P:256 + (s + 1) * P]
            bn_row = lambda s: a2_t[0:1, 512 + s * P:512 + (s + 1) * P]
            bh_row = lambda s: a2_t[0:1, 768 + s * P:768 + (s + 1) * P]
            W2 = lambda q: we2_t[:, q * 256:(q + 1) * 256]
            AdjT = lambda q: adjt_t[:, q * 256:(q + 1) * 256]
            Wn_ = lambda q, s: wn_t[:, q * 256 + s * P:q * 256 + (s + 1) * P]
            Wh_ = lambda q, s: wh_t[:, q * 256 + s * P:q * 256 + (s + 1) * P]
            Wl_ = lambda q, s: e_t[:, q * 256 + s * P:q * 256 + (s + 1) * P]
            M_ = e_t[:, 512:640]
            oh_ = lambda q: adjt_t[:, 512 + q:513 + q]
            Wbig = lambda s: e_t[:, 640 + s * 8:640 + (s + 1) * 8]

            ones_t = wp.tile([1, 256], BF16)
            nc.vector.memset(ones_t[:], 1.0)
            ones128 = ones_t[0:1, 0:P]
            ones1 = ones_t[0:1, 0:1]
            zeros_t = wp.tile([P, 256], F32)
            nc.vector.memset(zeros_t[:], 0.0)

            res_t = wp.tile([P, 1], F32)

            # PE warm-up: pins pe_busy_start early so the clock ramp finishes
            # before the real matmuls arrive.
            pwarm = psv.tile([1, 1], F32, tag="vec")
            nc.tensor.matmul(pwarm[:], ones1, ones1, start=True, stop=True)

            # ---- stage 1: E1T[t, n] = relu(We1.T @ obsT + be1) ------------
            # be1 folded in as contraction row 40 (obsT row 40 = ones).
            E1T = wp.tile([P, 2, 256], BF16)
            for s in range(2):
                pm = ps.tile([P, 256], F32, tag="mm")
                nc.tensor.matmul(pm[:], We1a(s), obsT, start=True, stop=True)
                if s == 0:
                    nc.scalar.activation(E1T[:, s, :], pm[:], AF.Relu)
                else:
                    nc.vector.tensor_scalar_max(E1T[:, s, :], pm[:], 0.0)

            # ---- stage 2: E[n, t'] = relu(E1 @ We2 + be2) -----------------
            E = wp.tile([P, 2, 256], BF16)
            for s in range(2):
                pm = ps.tile([P, 256], F32, tag="mm")
                nc.tensor.matmul(pm[:], E1T[:, 0, ts(s, P)], W2(0),
                                 start=True, stop=False)
                nc.tensor.matmul(pm[:], E1T[:, 1, ts(s, P)], W2(1),
                                 start=False, stop=False)
                nc.tensor.matmul(pm[:], ones128, be2_row,
                                 start=False, stop=True)
                if s == 0:
                    nc.scalar.activation(E[:, s, :], pm[:], AF.Relu)
                else:
                    nc.vector.tensor_scalar_max(E[:, s, :], pm[:], 0.0)

            # ---- stage 3: AET[t, m] = (adj @ E).T -------------------------
            AET = wp.tile([P, 2, 256], BF16)
            for s in range(2):
                pm = ps.tile([P, 256], F32, tag="mm")
                nc.tensor.matmul(pm[:], E[:, 0, ts(s, P)], AdjT(0),
                                 start=True, stop=False)
                nc.tensor.matmul(pm[:], E[:, 1, ts(s, P)], AdjT(1),
                                 start=False, stop=True)
                if s == 0:
                    nc.vector.tensor_copy(AET[:, s, :], pm[:])
                else:
                    nc.scalar.copy(AET[:, s, :], pm[:])

            # ---- stage 6: et = E[tgt, :] via one-hot ----------------------
            etps = psv.tile([P, 2], F32, tag="vec")
            for s in range(2):
                nc.tensor.matmul(etps[:, s:s + 1], E[:, 0, ts(s, P)], oh_(0),
                                 start=True, stop=False)
                nc.tensor.matmul(etps[:, s:s + 1], E[:, 1, ts(s, P)], oh_(1),
                                 start=False, stop=True)
            et = wp.tile([P, 2], BF16)
            nc.vector.tensor_copy(et[:], etps[:])

            # ---- stage 7: a = relu(Wl.T @ et + bl) ------------------------
            aps = psv.tile([P, 2], F32, tag="vec")
            for s in range(2):
                nc.tensor.matmul(aps[:, s:s + 1], Wl_(0, s), et[:, 0:1],
                                 start=True, stop=False)
                nc.tensor.matmul(aps[:, s:s + 1], Wl_(1, s), et[:, 1:2],
                                 start=False, stop=False)
                nc.tensor.matmul(aps[:, s:s + 1], bl_row(s), ones1,
                                 start=False, stop=True)
            a_t = wp.tile([P, 2], F32)
            nc.scalar.activation(a_t[:], aps[:], AF.Relu)

            # ---- stage 4: S_n[t'] = sum_m relu(Wn.T @ AET + bn) -----------
            Sn = wp.tile([P, 2], F32)
            Sh = wp.tile([P, 2], F32)

            def relu_rowsum(Wsl, brow, S_t, act_after=None, dve_after=None):
                for s in range(2):
                    pm = ps.tile([P, 256], F32, tag="mm")
                    nc.tensor.matmul(pm[:], Wsl(0, s), AET[:, 0, :],
                                     start=True, stop=False)
                    nc.tensor.matmul(pm[:], Wsl(1, s), AET[:, 1, :],
                                     start=False, stop=False)
                    nc.tensor.matmul(pm[:], brow(s), ones_t[0:1, 0:256],
                                     start=False, stop=True)
                    zt = work.tile([P, 256], BF16, tag="zt")
                    if s == 0:
                        bi = nc.scalar.activation(zt[:], pm[:], AF.Relu,
                                                  accum_out=S_t[:, 0:1])
                        if act_after is not None:
                            tile.add_dep_helper(act_after.ins, bi.ins,
                                                sync=True,
                                                reason="softmax before Sh")
                    else:
                        bi = nc.vector.scalar_tensor_tensor(
                            zt[:], pm[:], 0.0, zeros_t[:],
                            ALU.add, ALU.max, accum_out=S_t[:, 1:2])
                        if dve_after is not None:
                            tile.add_dep_helper(dve_after.ins, bi.ins,
                                                sync=True,
                                                reason="softmax before Sh")

            relu_rowsum(Wn_, bn_row, Sn)

            # ---- softmax epilogue in [T, 1] column layout -----------------
            l_t = wp.tile([P, 2], F32)
            expl = wp.tile([P, 2], BF16)
            nc.vector.tensor_tensor(l_t[:], a_t[:], Sn[:], ALU.mult)
            min_bi = nc.vector.tensor_scalar_min(l_t[:], l_t[:], CLAMP)
            exp_bi = nc.scalar.activation(expl[:], l_t[:], AF.Exp)

            # ---- stage 5: S_h (after Sn so softmax overlaps on ACT/DVE) ---
            relu_rowsum(Wh_, bh_row, Sh, act_after=exp_bi, dve_after=min_bi)

            # denom broadcast: denb[p] = sum_{t2} M[t2, p%...] expl[t2]
            denb = pss.tile([P, 1], F32, tag="den")
            nc.tensor.matmul(denb[:], M_, expl[:, 0:1], start=True, stop=False)
            nc.tensor.matmul(denb[:], M_, expl[:, 1:2], start=False, stop=True)
            recipb = wp.tile([P, 1], F32)
            nc.vector.reciprocal(recipb[:], denb[:])

            # v[t] = expl[t] * recip[t%8] * S_h[t]; act = Wbig.T @ v + ba
            v_t = wp.tile([P, 2], BF16)
            nc.vector.scalar_tensor_tensor(v_t[:], expl[:], recipb[:], Sh[:],
                                           ALU.mult, ALU.mult)
            pa = pss.tile([ACT, 1], F32, tag="den")
            nc.tensor.matmul(pa[:], Wbig(0), v_t[:, 0:1], start=True, stop=False)
            nc.tensor.matmul(pa[:], Wbig(1), v_t[:, 1:2], start=False, stop=False)
            nc.tensor.matmul(pa[:], ba_row, ones1, start=False, stop=True)
            nc.vector.tensor_copy(res_t[0:ACT, :], pa[:])
            nc.sync.dma_start(d_out.ap()[0:ACT], res_t[0:ACT, 0])

    nc.compile()
    return nc


def get_nc():
    if "nc" not in _CACHE:
        _CACHE["nc"] = _build_nc()
    return _CACHE["nc"]


def _pack2(W):
    """[256, 256] -> [128, 512] with [p, q*256+m] = W[q*128+p, m], bf16."""
    W = np.asarray(W, np.float32).astype(BF)
    return np.ascontiguousarray(W.reshape(2, P, 256).transpose(1, 0, 2).reshape(P, 512))


def make_in_maps(x, adj, We1, be1, We2, be2, Wl, bl, Wn, bn, Wh, bh, Wa, ba):
    f = lambda v: np.asarray(v, np.float32)
    bf = lambda v: np.asarray(v, np.float32).astype(BF)
    x = f(x)
    tgt = x[:, -1, 0].astype(np.int32)
    obs = x[:, :-1, :]

    a1_base = np.zeros((48, A1_COLS), BF)
    a1_base[40, 0:256] = BF(1.0)
    a1_base[0:40, 256:512] = bf(We1)
    a1_base[40, 256:512] = bf(be1)
    a1_base[0, 512:520] = bf(ba)

    a2 = np.zeros((1, A2_COLS), BF)
    a2[0, 0:256] = bf(be2)
    a2[0, 256:512] = bf(bl)
    a2[0, 512:768] = bf(bn)
    a2[0, 768:1024] = bf(bh)

    pk_we2 = _pack2(We2)
    adjt_base = np.zeros((P, ADJ_COLS), BF)
    adjt_base[:, 0:512] = _pack2(f(adj).T)
    pk_wn = _pack2(Wn)
    pk_wh = _pack2(Wh)

    e_pk = np.zeros((P, E_COLS), BF)
    e_pk[:, 0:512] = _pack2(Wl)
    e_pk[:, 512:640] = (np.arange(P)[:, None] % HEAD ==
                        np.arange(P)[None, :] % HEAD).astype(BF)
    Wa8 = f(Wa) / HEAD
    for s in range(2):
        rows = (s * P + np.arange(P)) // HEAD
        e_pk[:, 640 + s * 8:648 + s * 8] = bf(Wa8[rows, :])
    e_pk = np.ascontiguousarray(e_pk)

    in_maps = []
    for c in range(8):
        b = c % B
        a1 = a1_base.copy()
        a1[0:40, 0:256] = bf(obs[b].T)
        adjt = adjt_base.copy()
        q, r = divmod(int(tgt[b]), P)
        adjt[r, 512 + q] = BF(1.0)
        in_maps.append({
            "pk_a1": np.ascontiguousarray(a1),
            "pk_a2": a2,
            "pk_we2": pk_we2,
            "pk_adjt": np.ascontiguousarray(adjt),
            "pk_wn": pk_wn,
            "pk_wh": pk_wh,
            "pk_e": e_pk,
        })
    return in_maps


def run(in_maps, **kwargs):
    nc = get_nc()
    return bass_utils.run_bass_kernel_spmd(
        nc, in_maps, core_ids=list(range(8)), **kwargs)


def kernel(**inputs) -> np.ndarray:
    in_maps = make_in_maps(**inputs)
    res = run(in_maps)
    return np.stack(
        [res.results[b]["act_out"][:ACT] for b in range(B)], axis=0).astype(np.float32)
